# revision 2
# baseline (speedup 1.0000x reference)
"""Multi-head attention (B=8, N=1024, C=1024, H=16, D=64) with QK RMS-norm,
data-parallel across 8 NeuronCores. fp16 operands, f32 PSUM.

v2 of the software-pipelined mega-loop (vs v1):
 - warmup matmuls at t=0 pull the PE HAM clock-gate to K=8/8 during the
   initial DMA wait, so real matmuls start at full clock (~10us head win).
 - slot-0 DMA priority: pair-0 weights + x chunks first, wvs after.
 - V matmuls redistributed (jh0 slots 0-1, jh1 slots 3-4) as mid filler.
 - proj in three stages: ct0-3 (slot 8 filler), ct4-6 + ct7 + combine
   (slot 10), so the PE stays dense late and the tail shrinks; partials
   held in fp16 SBUF; projC alternates mm/bc banks to avoid DVE pacing.
 - wvs/wpst share one SBUF buffer (WAR-ordered via same pool tag).

PSUM budget (8 banks): mm 2, bc 1, sps 2x2, o 1.
"""

import numpy as np

import concourse.bacc as bacc
import concourse.bass as bass
import concourse.tile as tile
from concourse import mybir
from concourse.bass_utils import run_bass_kernel_spmd

F32 = mybir.dt.float32
F32R = mybir.dt.float32r
F16 = mybir.dt.float16
U16 = mybir.dt.uint16
AF = mybir.ActivationFunctionType
OP = mybir.AluOpType

B, N, C = 8, 1024, 1024
H, D = 16, 64
EPS = 1e-6
NCORES = 8
NT = N // 128
CT = C // 128
NPAIR = H // 2
NSLOT = NPAIR + 3


def _build():
    nc = bacc.Bacc(None, target_bir_lowering=False)

    xT_d = nc.dram_tensor("xT", [C, N], F16, kind="ExternalInput")
    wqkvT_d = nc.dram_tensor("wqkvT", [C, 3 * C], F16, kind="ExternalInput")
    wprojT_d = nc.dram_tensor("wprojT", [C, C], F16, kind="ExternalInput")
    bqkv_d = nc.dram_tensor("bqkv", [3 * C], F32, kind="ExternalInput")
    beff_d = nc.dram_tensor("beff", [C], F32, kind="ExternalInput")
    selq_d = nc.dram_tensor("selq", [8, 128], F16, kind="ExternalInput")
    selk_d = nc.dram_tensor("selk", [8, 128], F16, kind="ExternalInput")
    y_d = nc.dram_tensor("y", [N, C], F32, kind="ExternalOutput")

    with tile.TileContext(nc) as tc:
        with (
            tc.tile_pool(name="lp", bufs=1) as lp,
            tc.tile_pool(name="wp", bufs=3) as wp,
            tc.tile_pool(name="rawp", bufs=2) as rawp,
            tc.tile_pool(name="sqp", bufs=2) as sqp,
            tc.tile_pool(name="smp", bufs=3) as smp,
            tc.tile_pool(name="rpp", bufs=2) as rpp,
            tc.tile_pool(name="brp", bufs=1) as brp,
            tc.tile_pool(name="psbp", bufs=10) as psbp,
            tc.tile_pool(name="stagep", bufs=10) as stagep,
            tc.tile_pool(name="ysbp", bufs=3) as ysbp,
            tc.tile_pool(name="mm_ps", bufs=2, space="PSUM") as mm_ps,
            tc.tile_pool(name="bc_ps", bufs=1, space="PSUM") as bc_ps,
            tc.tile_pool(name="sps_ps", bufs=2, space="PSUM") as sps_ps,
            tc.tile_pool(name="o_ps", bufs=1, space="PSUM") as o_ps,
        ):
            # ---------------- persistent tiles ----------------
            xt = lp.tile([128, CT, N], F16, tag="xt", name="xt")
            # wvs (read last at slot 4) and wpst (loaded at slot 7) share one
            # 16KB/partition buffer via the same pool tag (WAR-ordered).
            wvs = lp.tile([128, CT, 1024], F16, tag="wsh", name="wvs")
            wpst = lp.tile([128, CT, 1024], F16, tag="wsh", name="wpst")
            qk = [lp.tile([128, N], F16, tag=f"qk{i}", name=f"qk{i}") for i in range(2 * NPAIR)]
            vaug = [lp.tile([128, H, D + 1], F16, tag=f"va{i}", name=f"va{i}") for i in range(NT)]
            attnT = [lp.tile([128, N], F16, tag=f"at{p}", name=f"at{p}") for p in range(NPAIR)]
            partial = [lp.tile([128, 512], F16, tag=f"pp{i}", name=f"pp{i}")
                       for i in range(16)]
            ones2 = lp.tile([128, 2], F16, tag="ones2", name="ones2")
            ones1 = lp.tile([1, 128], F32R, tag="ones1", name="ones1")
            selq = lp.tile([98, 128], F16, tag="selq", name="selq")
            selk = lp.tile([98, 128], F16, tag="selk", name="selk")
            bp_bc = lp.tile([128, C], F32, tag="bpbc", name="bpbc")
            dummy = lp.tile([128, 512], F16, tag="dummy", name="dummy")
            magic = lp.tile([128, 32], U16, tag="magic", name="magic")
            rq_sb = [lp.tile([98, N], F16, tag=f"rqs{g}", name=f"rqs{g}") for g in range(2)]
            rk_sb = [lp.tile([98, N], F16, tag=f"rks{g}", name=f"rks{g}") for g in range(2)]
            dv_sb = [lp.tile([98, N], F16, tag=f"dvs{g}", name=f"dvs{g}") for g in range(2)]

            xsrc = xT_d[:, :].rearrange("(ct p) n -> p ct n", p=128)

            # per-pair state carried between slots
            st_wts = {}     # p -> (bias_q, wts_q, bias_k, wts_k)
            st_raw = {}     # p -> (raw_q, raw_k)
            st_sq = {}      # p -> (sq_q, sq_k)
            st_rp = {}      # p -> (rp2q, rp2k)
            st_dr = {}      # p -> dr tile
            st_s65 = {}     # (p, e, nh) -> staging tile [65, 512] f16

            # ---------------- unit builders (closures) ----------------
            def u_const():
                nc.vector.memset(ones2.bitcast(U16), 0)
                nc.vector.memset(ones2[0:64, 0:1].bitcast(U16), 0x3C00)
                nc.vector.memset(ones2[64:128, 1:2].bitcast(U16), 0x3C00)
                nc.vector.memset(ones1.bitcast(F32), 1.0)
                nc.vector.memset(dummy.bitcast(U16), 0x3C00)
                nc.vector.memset(magic, 0x59B8)
                for j in range(4):
                    nc.sync.dma_start(out=selq[32 * j:32 * j + 2, :],
                                      in_=selq_d[2 * j:2 * j + 2, :])
                    nc.sync.dma_start(out=selk[32 * j:32 * j + 2, :],
                                      in_=selk_d[2 * j:2 * j + 2, :])
                for nt in range(NT):
                    nc.vector.memset(vaug[nt][:, :, D:D + 1].bitcast(U16), 0x3C00)

            def u_warm(g):
                # dense dummy matmuls at t=0: pull HAM to K=8/8 during the
                # initial DMA wait so real matmuls start at full clock
                def run():
                    wt = mm_ps.tile([128, 512], F32, tag="mm", name=f"warm{g}")
                    for i in range(8):
                        nc.tensor.matmul(out=wt[:, :], lhsT=dummy[:, 0:128],
                                         rhs=dummy[:, :],
                                         start=(i == 0), stop=(i == 7))
                return run

            brow_g = [None]

            def u_beff():
                brow = brp.tile([1, C], F32R, tag="brow", name="brow")
                brow_g[0] = brow
                nc.sync.dma_start(out=brow, in_=beff_d[:].unsqueeze(0).bitcast(F32R))
                for half in range(2):
                    hs = slice(half * 512, (half + 1) * 512)
                    bb = bc_ps.tile([128, 512], F32, tag="bc", name=f"bb{half}")
                    nc.tensor.matmul(out=bb[:, :], lhsT=ones1[:, :], rhs=brow[:, hs],
                                     start=True, stop=True)
                    nc.vector.tensor_copy(out=bp_bc[:, hs], in_=bb[:, :])

            def u_load_w(p):
                def run():
                    tiles = []
                    for jt in (p, NPAIR + p):
                        bias_c = wp.tile([128, 1], F32, tag="biasc", name=f"bc{jt}")
                        nc.sync.dma_start(out=bias_c,
                                          in_=bqkv_d[jt * 128:(jt + 1) * 128].unsqueeze(1))
                        wts = wp.tile([128, CT, 128], F16, tag="wts", name=f"wts{jt}")
                        nc.sync.dma_start(
                            out=wts,
                            in_=wqkvT_d[:, jt * 128:(jt + 1) * 128]
                            .rearrange("(ct p) j -> p ct j", p=128))
                        tiles += [bias_c, wts]
                    st_wts[p] = tiles
                return run

            def u_qkv(p, which, nh, chalf):
                # one half of the ct-accumulation for q (which=0) or k (which=1)
                def run():
                    jt = p + NPAIR * which
                    key = (p, which, nh)
                    if chalf == 0:
                        ps = mm_ps.tile([128, 512], F32, tag="mm", name=f"qk{jt}h{nh}")
                        u_qkv.ps[key] = ps
                    else:
                        ps = u_qkv.ps.pop(key)
                    bias_c, wts = st_wts[p][2 * which:2 * which + 2]
                    for ct in range(4 * chalf, 4 * chalf + 4):
                        nc.tensor.matmul(
                            out=ps[:, :], lhsT=wts[:, ct, :],
                            rhs=xt[:, ct, nh * 512:(nh + 1) * 512],
                            start=(ct == 0), stop=(ct == CT - 1))
                    if chalf == 1:
                        if nh == 0 and which == 0:
                            raw_q = rawp.tile([128, N], F16, tag="rawq", name=f"rawq{p}")
                            raw_k = rawp.tile([128, N], F16, tag="rawk", name=f"rawk{p}")
                            st_raw[p] = (raw_q, raw_k)
                        raw = st_raw[p][which]
                        nc.vector.tensor_scalar(
                            out=raw[:, nh * 512:(nh + 1) * 512],
                            in0=ps[:, :], scalar1=bias_c[:, :], scalar2=None, op0=OP.add)
                        if nh == 0 and which == 0:
                            st_sq[p] = [None, None]
                        if st_sq[p][which] is None:
                            sq = sqp.tile([128, N], F16, tag="sqq" if which == 0 else "sqk",
                                          name=f"sq{jt}")
                            st_sq[p][which] = sq
                        sq = st_sq[p][which]
                        nsl = slice(nh * 512, (nh + 1) * 512)
                        nc.vector.tensor_tensor(out=sq[:, nsl], in0=raw[:, nsl],
                                                in1=raw[:, nsl], op=OP.mult)
                return run
            u_qkv.ps = {}

            def u_v(nt, jh, chalf):
                def run():
                    key = (nt, jh)
                    if chalf == 0:
                        ps = mm_ps.tile([128, 512], F32, tag="mm", name=f"v{nt}h{jh}")
                        u_v.ps[key] = ps
                    else:
                        ps = u_v.ps.pop(key)
                    for ct in range(4 * chalf, 4 * chalf + 4):
                        nc.tensor.matmul(
                            out=ps[:, :], lhsT=xt[:, ct, nt * 128:(nt + 1) * 128],
                            rhs=wvs[:, ct, jh * 512:(jh + 1) * 512],
                            start=(ct == 0), stop=(ct == CT - 1))
                    if chalf == 1:
                        dst = vaug[nt][:, jh * 8:(jh + 1) * 8, 0:D]
                        nc.scalar.activation(
                            out=dst, in_=ps[:, :].rearrange("p (h d) -> p h d", d=D),
                            func=AF.Copy)
                return run
            u_v.ps = {}

            def u_ssq(p, which):
                def run():
                    sq = st_sq[p][which]
                    ssqt = mm_ps.tile([128, 512], F32, tag="mm", name=f"ssq{p}w{which}")
                    rp = rpp.tile([2, N], F16, tag="rp2q" if which == 0 else "rp2k",
                                  name=f"rp2{p}w{which}")
                    for nh in range(2):
                        nc.tensor.matmul(
                            out=ssqt[32 * nh:32 * nh + 2, :], lhsT=ones2[:, :],
                            rhs=sq[:, nh * 512:(nh + 1) * 512],
                            start=True, stop=True)
                    for nh in range(2):
                        nc.vector.tensor_scalar(
                            out=rp[:, nh * 512:(nh + 1) * 512],
                            in0=ssqt[32 * nh:32 * nh + 2, :],
                            scalar1=1.0 / D, scalar2=EPS, op0=OP.mult, op1=OP.add)
                    if which == 0:
                        st_rp[p] = [rp, None]
                    else:
                        st_rp[p][1] = rp
                        st_sq.pop(p)
                return run

            def u_chain(p):
                # reshape DMA -> rsqrt newton -> scatter into selector layout
                def run():
                    g, jj = divmod(p, 4)
                    rp2q, rp2k = st_rp.pop(p)
                    rrp = smp.tile([128, 32], F16, tag="rrp", name=f"rrp{p}")
                    ro = smp.tile([128, 32], F16, tag="ro", name=f"ro{p}")
                    t2 = smp.tile([128, 32], F16, tag="t2", name=f"t2{p}")
                    nc.sync.dma_start(out=rrp[0:64, :], in_=rp2q[:, :])
                    nc.sync.dma_start(out=rrp[64:128, :], in_=rp2k[:, :])
                    with nc.allow_low_precision(reason="fp16 rsqrt newton, 8e-4 rel"):
                        nc.vector.tensor_scalar(out=ro.bitcast(U16), in0=rrp.bitcast(U16),
                                                scalar1=1, scalar2=None,
                                                op0=OP.logical_shift_right)
                        nc.vector.tensor_tensor(out=ro.bitcast(U16), in0=magic.bitcast(U16),
                                                in1=ro.bitcast(U16), op=OP.subtract)
                        for _ in range(2):
                            nc.vector.tensor_tensor(out=t2, in0=ro, in1=ro, op=OP.mult)
                            nc.vector.tensor_tensor(out=t2, in0=t2, in1=rrp, op=OP.mult)
                            nc.vector.tensor_scalar(out=t2, in0=t2, scalar1=-0.5, scalar2=1.5,
                                                    op0=OP.mult, op1=OP.add)
                            nc.vector.tensor_tensor(out=ro, in0=ro, in1=t2, op=OP.mult)
                    nc.sync.dma_start(out=rq_sb[g][32 * jj:32 * jj + 2, :],
                                      in_=ro[0:64, :])
                    nc.sync.dma_start(out=rk_sb[g][32 * jj:32 * jj + 2, :],
                                      in_=ro[64:128, :])
                return run

            def u_bcast(p, which, nh):
                # selector-matmul broadcast of 1/rms + scale-multiply -> qk16
                def run():
                    g, j = divmod(p, 4)
                    jt = p + NPAIR * which
                    sel = selq if which == 0 else selk
                    r_sb = rq_sb[g] if which == 0 else rk_sb[g]
                    nsl = slice(nh * 512, (nh + 1) * 512)
                    bc = bc_ps.tile([128, 512], F32, tag="bc", name=f"r{jt}h{nh}")
                    nc.tensor.matmul(out=bc[:, :], lhsT=sel[32 * j:32 * j + 2, :],
                                     rhs=r_sb[32 * j:32 * j + 2, nsl],
                                     start=True, stop=True, tile_position=(32 * j, 0))
                    raw = st_raw[p][which]
                    nc.vector.tensor_tensor(out=qk[jt][:, nsl], in0=raw[:, nsl],
                                            in1=bc[:, :], op=OP.mult)
                    if which == 1 and nh == 1:
                        st_raw.pop(p)
                return run

            def u_norm(p, nh):
                def run():
                    g, j = divmod(p, 4)
                    nsl = slice(nh * 512, (nh + 1) * 512)
                    dbp = bc_ps.tile([128, 512], F32, tag="bc", name=f"db{p}h{nh}")
                    nc.tensor.matmul(out=dbp[:, :], lhsT=selq[32 * j:32 * j + 2, :],
                                     rhs=dv_sb[g][32 * j:32 * j + 2, nsl],
                                     start=True, stop=True, tile_position=(32 * j, 0))
                    for e in range(2):
                        s65 = st_s65.pop((p, e, nh))
                        nc.vector.tensor_tensor(
                            out=attnT[p][64 * e:64 * e + 64, nsl], in0=s65[0:D, :],
                            in1=dbp[64 * e:64 * e + 64, :], op=OP.mult)
                return run

            def u_dma(fn):
                def run():
                    fn()
                return run

            # -------- proj in three stages: ct0-3, ct4-6, ct7+combine --------
            def u_projA(ch, nt):
                # stage 1 (slot 8 filler): ct 0..3 -> partial (f16, +bias)
                def run():
                    yp = mm_ps.tile([128, 512], F32, tag="mm", name=f"ya{ch}{nt}")
                    for ct in range(4):
                        nc.tensor.matmul(
                            out=yp[:, :],
                            lhsT=attnT[ct][:, nt * 128:(nt + 1) * 128],
                            rhs=wpst[:, ct, ch * 512:(ch + 1) * 512],
                            start=(ct == 0), stop=(ct == 3))
                    nc.vector.tensor_tensor(
                        out=partial[ch * 8 + nt], in0=yp[:, :],
                        in1=bp_bc[:, ch * 512:(ch + 1) * 512], op=OP.add)
                return run

            def u_projB(ch, nt):
                # stage 2 (slot 10 filler): ct 4..6 -> partial += psum
                def run():
                    yp = mm_ps.tile([128, 512], F32, tag="mm", name=f"yb{ch}{nt}")
                    for ct in range(4, 7):
                        nc.tensor.matmul(
                            out=yp[:, :],
                            lhsT=attnT[ct][:, nt * 128:(nt + 1) * 128],
                            rhs=wpst[:, ct, ch * 512:(ch + 1) * 512],
                            start=(ct == 4), stop=(ct == 6))
                    nc.vector.tensor_tensor(
                        out=partial[ch * 8 + nt], in0=yp[:, :],
                        in1=partial[ch * 8 + nt], op=OP.add)
                return run

            def u_projC(ch, nt):
                # tail: ct 7 + final combine + store (alternate psum banks so
                # the PE isn't paced by a single pool's DVE drain)
                def run():
                    pool, tg = (mm_ps, "mm") if (ch * 8 + nt) % 2 == 0 else (bc_ps, "bc")
                    yp = pool.tile([128, 512], F32, tag=tg, name=f"yc{ch}{nt}")
                    nc.tensor.matmul(
                        out=yp[:, :],
                        lhsT=attnT[7][:, nt * 128:(nt + 1) * 128],
                        rhs=wpst[:, 7, ch * 512:(ch + 1) * 512],
                        start=True, stop=True)
                    ysb = ysbp.tile([128, 512], F32, tag="ysb", name="ysb")
                    nc.vector.tensor_tensor(out=ysb, in0=yp[:, :],
                                            in1=partial[ch * 8 + nt], op=OP.add)
                    nc.sync.dma_start(
                        out=y_d[nt * 128:(nt + 1) * 128, ch * 512:(ch + 1) * 512],
                        in_=ysb)
                return run

            # ---------------- attention generator ----------------
            def gen_attn(p):
                # per nh: S pair matmuls + one exp per st; head-even O
                # accumulates in-loop (split-K halves into two banks),
                # head-odd O runs as a dense second pass
                dr = smp.tile([128, 16], F16, tag="dr", name=f"dr{p}")
                st_dr[p] = dr
                for nh in range(2):
                    nsl = slice(nh * 512, (nh + 1) * 512)
                    sps_l = {}
                    psb_l = {}

                    def emit_S(st):
                        sps = sps_ps.tile([128, 2, 512], F32, tag="sp",
                                          name=f"s{p}n{nh}t{st}")
                        for e in range(2):
                            nc.tensor.matmul(
                                out=sps[:, e, :],
                                lhsT=qk[NPAIR + p][64 * e:64 * e + 64, st * 128:(st + 1) * 128],
                                rhs=qk[p][64 * e:64 * e + 64, nsl],
                                start=True, stop=True, tile_position=(64 * e, 0))
                        sps_l[st] = sps

                    def emit_exp(st):
                        psb = psbp.tile([128, 2, 512], F16, tag="psb",
                                        name=f"p{p}n{nh}t{st}")
                        nc.scalar.activation(out=psb, in_=sps_l.pop(st), func=AF.Exp)
                        psb_l[st] = psb

                    def emit_O(e, st, o):
                        nc.tensor.matmul(
                            out=o[:, :], lhsT=vaug[st][:, 2 * p + e, :],
                            rhs=psb_l[st][:, e, :],
                            start=(st == 0), stop=(st == NT - 1))

                    def drain(e, o):
                        s65 = stagep.tile([65, 512], F16, tag="s65",
                                          name=f"s65e{e}{p}n{nh}")
                        nc.scalar.activation(out=s65, in_=o[:, :], func=AF.Copy)
                        st_s65[(p, e, nh)] = s65
                        off = 64 * e + 32 * nh
                        nc.sync.dma_start(out=dr[off:off + 32, :],
                                          in_=s65[D:D + 1, :])

                    o = o_ps.tile([D + 1, 512], F32, tag="o", name=f"oe{p}n{nh}")
                    emit_S(0)
                    emit_exp(0)
                    yield
                    for st in range(1, NT):
                        emit_S(st)
                        emit_exp(st)
                        yield
                        if st >= 2:
                            emit_O(0, st - 2, o)
                    emit_O(0, NT - 2, o)
                    emit_O(0, NT - 1, o)
                    yield
                    drain(0, o)
                    yield
                    # dense second pass: head-odd O over the retained psb tiles
                    # (o_ps bufs=2 -> doesn't wait for the even drain)
                    o2 = o_ps.tile([D + 1, 512], F32, tag="o", name=f"oo{p}n{nh}")
                    for st in range(NT):
                        emit_O(1, st, o2)
                    yield
                    drain(1, o2)
                    psb_l.clear()
                    yield
                # after both nh streams: reciprocal of denominators
                g, j = divmod(p, 4)
                dr2 = smp.tile([128, 16], F16, tag="dr2", name=f"dr2_{p}")
                with nc.allow_low_precision(reason="fp16 denom recip, 5e-4 rel err ok"):
                    nc.vector.reciprocal(out=dr2, in_=dr)
                nc.sync.dma_start(out=dv_sb[g][32 * j:32 * j + 2, :],
                                  in_=dr2[:, :])
                st_dr.pop(p)
                yield

            # ---------------- slot schedule ----------------
            def slot_units(s):
                spacers = []   # independent PE work (qkv, v, dma, proj stages)
                ordered = []   # mm-bank users in dependency order
                if s == 0:
                    spacers.append(u_const)
                    for g in range(3):
                        spacers.append(u_warm(g))
                    spacers.append(u_load_w(0))
                    for ci in range(8):
                        spacers.append(u_dma(lambda ci=ci: nc.sync.dma_start(
                            out=xt[:, ci:ci + 1, :], in_=xsrc[:, ci:ci + 1, :])))

                if s < NPAIR:
                    p = s
                    if p + 1 < NPAIR:
                        spacers.append(u_load_w(p + 1))
                    for which in range(2):
                        for nh in range(2):
                            for chalf in range(2):
                                spacers.append(u_qkv(p, which, nh, chalf))
                if s == 0:
                    spacers.append(u_dma(lambda: nc.sync.dma_start(
                        out=wvs[:, :, 0:512],
                        in_=wqkvT_d[:, 2 * C:2 * C + 512].rearrange("(ct p) j -> p ct j", p=128))))
                    for nt in range(4):
                        for chalf in range(2):
                            spacers.append(u_v(nt, 0, chalf))
                if s == 1:
                    spacers.append(u_dma(lambda: nc.sync.dma_start(
                        out=wvs[:, :, 512:1024],
                        in_=wqkvT_d[:, 2 * C + 512:3 * C].rearrange("(ct p) j -> p ct j", p=128))))
                    for nt in range(4, NT):
                        for chalf in range(2):
                            spacers.append(u_v(nt, 0, chalf))
                if s in (3, 4):
                    for nt in range(4 * (s - 3), 4 * (s - 3) + 4):
                        for chalf in range(2):
                            spacers.append(u_v(nt, 1, chalf))
                if s == 6:
                    spacers.append(u_beff)
                if 1 <= s <= NPAIR:
                    p = s - 1
                    ordered.append(("ssq", u_ssq(p, 0)))
                    ordered.append(("ssq", u_ssq(p, 1)))
                    ordered.append(("chain", u_chain(p)))
                    for which in range(2):
                        for nh in range(2):
                            ordered.append(("bc", u_bcast(p, which, nh)))
                if s == 7:
                    spacers.append(u_dma(lambda: nc.sync.dma_start(
                        out=wpst[:, :, 0:512],
                        in_=wprojT_d[:, 0:512].rearrange("(ct p) j -> p ct j", p=128))))
                if s == 8:
                    spacers.append(u_dma(lambda: nc.sync.dma_start(
                        out=wpst[:, :, 512:1024],
                        in_=wprojT_d[:, 512:1024].rearrange("(ct p) j -> p ct j", p=128))))
                    # stage-1 proj filler: attnT[0..3] are done (norm(3) @ slot 6)
                    for nt in range(NT):
                        spacers.append(u_projA(0, nt))
                if s == 9:
                    for nt in range(NT):
                        spacers.append(u_projA(1, nt))
                if s == 10:
                    # stage-2 proj filler: attnT[4..6] done (norm(6) @ slot 9)
                    for ch in range(2):
                        for nt in range(NT):
                            spacers.append(u_projB(ch, nt))

                if 0 <= s - 3 < NPAIR:
                    # norms first: dv_sb is ready from the previous slot, so
                    # these are immediately-runnable PE work
                    for nh in range(2):
                        ordered.append(("bc", u_norm(s - 3, nh)))
                if s == 10:
                    # tail combine: must come after norm(7) writes attnT[7]
                    for ch in range(2):
                        for nt in range(NT):
                            ordered.append(("bc", u_projC(ch, nt)))

                return spacers + [u for _, u in ordered]

            for s in range(NSLOT):
                units = slot_units(s)
                if s >= 2 and s - 2 < NPAIR:
                    gen = gen_attn(s - 2)
                    yields = 2 * (NT + 4) + 1
                    done = 0
                    i = 0
                    for _ in gen:
                        i += 1
                        target = min(len(units), i * len(units) // yields + 1)
                        while done < target:
                            units[done]()
                            done += 1
                    while done < len(units):
                        units[done]()
                        done += 1
                else:
                    for u in units:
                        u()


    nc.compile()
    return nc


_NC = None


def _get_nc():
    global _NC
    if _NC is None:
        _NC = _build()
    return _NC


def make_in_maps(x, w_qkv, b_qkv, qn_w, kn_w, w_proj, b_proj):
    x = np.asarray(x, dtype=np.float32)
    xT = np.ascontiguousarray(np.transpose(x, (0, 2, 1)).astype(np.float16))
    wqkvT = np.ascontiguousarray(np.asarray(w_qkv, np.float32).T.astype(np.float16))
    wprojT = np.ascontiguousarray(np.asarray(w_proj, np.float32).T.astype(np.float16))
    w_proj32 = np.asarray(w_proj, np.float32)
    b_v = np.asarray(b_qkv, np.float32)[2 * C:3 * C]
    beff = np.asarray(b_proj, np.float32) + w_proj32 @ b_v
    scale = np.float32(1.0) / np.sqrt(np.float32(D)).astype(np.float32)
    qnkn = (np.asarray(qn_w, np.float32) * np.asarray(kn_w, np.float32) * scale)
    selq = np.zeros((8, 128), np.float16)
    selk = np.zeros((8, 128), np.float16)
    for g in range(4):
        selq[2 * g, 0:64] = 1.0
        selq[2 * g + 1, 64:128] = 1.0
        selk[2 * g, 0:64] = qnkn
        selk[2 * g + 1, 64:128] = qnkn
    return [
        {
            "xT": xT[b],
            "wqkvT": wqkvT,
            "wprojT": wprojT,
            "bqkv": np.asarray(b_qkv, np.float32),
            "beff": beff,
            "selq": selq,
            "selk": selk,
        }
        for b in range(B)
    ]


def kernel(x, w_qkv, b_qkv, qn_w, kn_w, w_proj, b_proj, **_ignored):
    nc = _get_nc()
    in_maps = make_in_maps(x, w_qkv, b_qkv, qn_w, kn_w, w_proj, b_proj)
    res = run_bass_kernel_spmd(nc, in_maps, core_ids=list(range(NCORES)))
    return np.stack([res.results[b]["y"] for b in range(B)]).astype(np.float32)
